# revision 13
# baseline (speedup 1.0000x reference)
"""Trainium2 Bass kernel for nn_AutoregressiveNetwork (MADE-style dense MLP).

Data-parallel over the batch: 8 NeuronCores, 2048 rows each. All 64
per-dimension subnetworks (net0 folded in as {W1=0, B1=w0[0]+b0, W2=I,
W3=v0, B3=c0}) run in feature-major layout (x.T on SBUF partitions):

  L1 (float32r): two concurrent row-tiled matmuls, each K=64 / M=128
      computing two nets at once (nets share the streamed x.T).
  L2 (float32r): block-diagonal K=128 / M=128 matmuls, two nets per
      instruction.
  L3 (float16):  two-net block-diag K=128 / M=32 (zero-padded) matmuls,
      col-tiled across the four 32-column groups -> four (net, scale/
      trans) rows per group land in one PSUM bank.

float32r streams one column/cycle (4x faster than fp32) but requires
the matmul destination partition base to be 0, hence this structure.
ReLU+bias is fused into the PSUM->SBUF evacuations, split between
ScalarE (activation) and VectorE (tensor_scalar add+max) to balance
the two engines that can read PSUM.
"""
import numpy as np

from concourse import bacc, tile, mybir
from concourse.bass_utils import run_bass_kernel_spmd

DIM = 64
HID = 64
BATCH = 16384
NCORES = 8
BL = BATCH // NCORES          # 2048 batch rows per core
NT = 512                      # free-dim per matmul (one fp32 PSUM bank)
T = BL // NT                  # batch tiles per core
G = 16                        # groups of 4 nets
F32 = mybir.dt.float32
F32R = mybir.dt.float32r
F16 = mybir.dt.float16

TRACE = False                 # no NTFF hook in this container
_cache = {}


def _build(reps=1):
    key = ("nc", reps)
    if key in _cache:
        return _cache[key]
    nc = bacc.Bacc("TRN2", target_bir_lowering=False, debug=False,
                   num_devices=NCORES)

    xT2 = nc.declare_dram_parameter("xT2", [128, BL], F32R, isOutput=False)
    lw1 = nc.declare_dram_parameter("lw1", [128, G * 128], F32R, isOutput=False)
    lw2 = nc.declare_dram_parameter("lw2", [128, G * 256], F32R, isOutput=False)
    lw3 = nc.declare_dram_parameter("lw3", [128, 32 * 32], F16, isOutput=False)
    bb2 = nc.declare_dram_parameter("bb2", [128, 2 * G], F32, isOutput=False)
    bb3 = nc.declare_dram_parameter("bb3", [128, 8], F32, isOutput=False)
    out = nc.declare_dram_parameter("out", [16, 8 * BL], F32, isOutput=True)

    Relu = mybir.ActivationFunctionType.Relu
    ADD = mybir.AluOpType.add
    MAX = mybir.AluOpType.max

    with tile.TileContext(nc) as tc:
        with (
            tc.tile_pool(name="const", bufs=1) as cpool,
            tc.tile_pool(name="act", bufs=6) as apool,
            tc.tile_pool(name="ps1", bufs=2, space="PSUM") as pspool1,
            tc.tile_pool(name="ps2", bufs=3, space="PSUM") as pspool2,
            tc.tile_pool(name="ps3", bufs=1, space="PSUM") as pspool3,
        ):
            xw = cpool.tile([128, BL], F32R)
            w1 = cpool.tile([128, G * 128], F32R)
            w2 = cpool.tile([128, G * 256], F32R)
            w3 = cpool.tile([128, 32 * 32], F16)
            b2 = cpool.tile([128, 2 * G], F32)
            b3 = cpool.tile([128, 8], F32)
            osb = cpool.tile([128, 8 * BL], F32)
            # chunked loads so group-0 compute starts after ~1/8 of the
            # weights have landed
            for sb, dr in ((b2, bb2), (b3, bb3), (w3, lw3)):
                nc.sync.dma_start(sb[:], dr[:])
            for i in range(8):
                nc.sync.dma_start(xw[:, i * 256:(i + 1) * 256],
                                  xT2[:, i * 256:(i + 1) * 256])
                nc.sync.dma_start(w1[:, i * 256:(i + 1) * 256],
                                  lw1[:, i * 256:(i + 1) * 256])
                nc.sync.dma_start(w2[:, i * 512:(i + 1) * 512],
                                  lw2[:, i * 512:(i + 1) * 512])

            for _rep in range(reps):
              for t in range(T):
                xs = xw[:, t * NT:(t + 1) * NT]
                P3 = None
                for g in range(G):
                    q, gg = g // 2, g % 2
                    c1 = g * 128
                    c2 = g * 256

                    # ---- L1: h1 = relu(x @ W1m + b1); bias rides the
                    # ones-row of xT2 (x col 63 is unused by every net), so
                    # the evacuation is a bias-free relu over both banks
                    P1 = pspool1.tile([128, 2 * NT], F32, tag="ps1")
                    nc.tensor.matmul(P1[:, 0:NT], w1[0:64, c1:c1 + 128],
                                     xs[0:64, :], tile_position=(0, 0))
                    nc.tensor.matmul(P1[:, NT:2 * NT], w1[64:128, c1:c1 + 128],
                                     xs[64:128, :], tile_position=(64, 0))

                    s12 = apool.tile([128, 2 * NT], F32R, tag="s12")
                    nc.scalar.activation(s12[:], P1[:], Relu, bias=0.0)

                    # ---- L2: h2 = relu(h1 @ W2 + b2), block-diag 2 nets
                    P2a = pspool2.tile([128, NT], F32, tag="ps2")
                    P2b = pspool2.tile([128, NT], F32, tag="ps2")
                    nc.tensor.matmul(P2a[:], w2[:, c2:c2 + 128], s12[:, 0:NT],
                                     tile_position=(0, 0))
                    nc.tensor.matmul(P2b[:], w2[:, c2 + 128:c2 + 256],
                                     s12[:, NT:2 * NT], tile_position=(0, 0))

                    t1 = apool.tile([128, NT], F16, tag="t1")
                    t2 = apool.tile([128, NT], F16, tag="t2")
                    nc.vector.tensor_scalar(t1[:], P2a[:], b2[:, 2 * g:2 * g + 1],
                                            0.0, ADD, MAX)
                    nc.vector.tensor_scalar(t2[:], P2b[:], b2[:, 2 * g + 1:2 * g + 2],
                                            0.0, ADD, MAX)

                    # ---- L3: out = h2 @ W3 + b3 (fp16, col-tiled)
                    if gg == 0:
                        P3 = pspool3.tile([128, NT], F32, tag="ps3")
                    cg = 2 * gg          # col groups 0,1 for even g; 2,3 odd
                    w3c = q * 128 + cg * 32
                    nc.tensor.matmul(P3[32 * cg:32 * cg + 32, :],
                                     w3[:, w3c:w3c + 32], t1[:],
                                     tile_position=(0, 32 * cg))
                    nc.tensor.matmul(P3[32 * cg + 32:32 * cg + 64, :],
                                     w3[:, w3c + 32:w3c + 64], t2[:],
                                     tile_position=(0, 32 * cg + 32))

                    if gg == 1:
                        off = q * BL + t * NT
                        if (t * 8 + q) % 4 == 3:
                            nc.vector.tensor_scalar(osb[:, off:off + NT], P3[:],
                                                    b3[:, q:q + 1], None, ADD)
                        else:
                            nc.scalar.activation(
                                osb[:, off:off + NT], P3[:],
                                mybir.ActivationFunctionType.Identity,
                                bias=b3[:, q:q + 1])
                        if _rep == reps - 1:
                            for c in range(4):
                                nc.sync.dma_start(
                                    out[4 * c:4 * c + 4, off:off + NT],
                                    osb[32 * c:32 * c + 4, off:off + NT])


    nc.compile()
    _cache[key] = nc
    return nc


def _pair_of(q, c):
    g = 2 * q + c // 2
    return (4 * g, 4 * g + 1) if c % 2 == 0 else (4 * g + 2, 4 * g + 3)


def _pack_weights(w0, b0, v0, c0, W1, B1, W2, B2, W3, B3):
    f = np.float32
    # 64 nets in device order; net 0 is the constant network.
    W1n = np.zeros((64, DIM, HID), f)
    B1n = np.zeros((64, HID), f)
    W2n = np.zeros((64, HID, HID), f)
    B2n = np.zeros((64, HID), f)
    W3n = np.zeros((64, HID, 2), f)
    B3n = np.zeros((64, 2), f)

    mask = (np.arange(DIM)[None, :] < np.arange(1, DIM)[:, None]).astype(f)
    W1n[1:] = W1 * mask[:, :, None]
    B1n[1:] = B1
    W2n[1:] = W2
    B2n[1:] = B2
    W3n[1:] = W3
    B3n[1:] = B3
    # net 0: Linear(1,H)->ReLU->Linear(H,2) with constant ones input
    B1n[0] = w0[0] + b0
    W2n[0] = np.eye(HID, dtype=f)
    W3n[0] = v0
    B3n[0] = c0

    lw1 = np.zeros((128, G * 128), f)
    lw2 = np.zeros((128, G * 256), f)
    bb2 = np.zeros((128, 2 * G), f)
    for g in range(G):
        n = 4 * g
        c1 = g * 128
        c2 = g * 256
        # L1: [W1 n | W1 n+1] on partitions 0-63, [W1 n+2 | W1 n+3] on
        # 64-127; W1 row 63 is zero for every net (autoregressive mask), so
        # it carries the L1 bias against the ones-row of xT2
        lw1[0:64, c1:c1 + 64] = W1n[n]
        lw1[0:64, c1 + 64:c1 + 128] = W1n[n + 1]
        lw1[64:128, c1:c1 + 64] = W1n[n + 2]
        lw1[64:128, c1 + 64:c1 + 128] = W1n[n + 3]
        lw1[63, c1:c1 + 128] = np.concatenate([B1n[n], B1n[n + 1]])
        lw1[127, c1:c1 + 128] = np.concatenate([B1n[n + 2], B1n[n + 3]])
        # L2: block-diag pairs
        lw2[0:64, c2:c2 + 64] = W2n[n]
        lw2[64:128, c2 + 64:c2 + 128] = W2n[n + 1]
        lw2[0:64, c2 + 128:c2 + 192] = W2n[n + 2]
        lw2[64:128, c2 + 192:c2 + 256] = W2n[n + 3]
        bb2[:, 2 * g] = np.concatenate([B2n[n], B2n[n + 1]])
        bb2[:, 2 * g + 1] = np.concatenate([B2n[n + 2], B2n[n + 3]])

    lw3 = np.zeros((128, 32 * 32), np.float16)
    bb3 = np.zeros((128, 8), f)
    for q in range(8):
        for c in range(4):
            p0, p1 = _pair_of(q, c)
            col = q * 128 + c * 32
            lw3[0:64, col:col + 2] = W3n[p0]
            lw3[64:128, col + 2:col + 4] = W3n[p1]
            bb3[32 * c:32 * c + 4, q] = [B3n[p0, 0], B3n[p0, 1],
                                         B3n[p1, 0], B3n[p1, 1]]
    return dict(lw1=lw1, lw2=lw2, lw3=lw3, bb2=bb2, bb3=bb3)


def kernel(x, w0, b0, v0, c0, W1, B1, W2, B2, W3, B3):
    x = np.asarray(x, np.float32)
    args = [np.asarray(a, np.float32) for a in (w0, b0, v0, c0, W1, B1, W2, B2,
                                                W3, B3)]
    wdict = _pack_weights(*args)

    nc = _build()
    in_maps = []
    for core in range(NCORES):
        xT = np.ascontiguousarray(x[core * BL:(core + 1) * BL].T)   # [64, BL]
        xT2 = np.concatenate([xT, xT], axis=0)
        xT2[63, :] = 1.0          # ones-row carries the L1 bias
        xT2[127, :] = 1.0
        in_maps.append({"xT2": xT2, **wdict})

    res = run_bass_kernel_spmd(nc, in_maps, core_ids=list(range(NCORES)),
                               trace=TRACE)
    kernel.last_exec_time_ns = res.exec_time_ns

    scales = np.empty((BATCH, DIM), np.float32)
    trans = np.empty((BATCH, DIM), np.float32)
    for core in range(NCORES):
        oc = res.results[core]["out"]                # [16, 8*BL]
        r0 = core * BL
        for q in range(8):
            blk = oc[:, q * BL:(q + 1) * BL]         # [16, BL]
            for c in range(4):
                p0, p1 = _pair_of(q, c)
                scales[r0:r0 + BL, p0] = blk[4 * c + 0]
                trans[r0:r0 + BL, p0] = blk[4 * c + 1]
                scales[r0:r0 + BL, p1] = blk[4 * c + 2]
                trans[r0:r0 + BL, p1] = blk[4 * c + 3]

    np.clip(scales, -5.0, 5.0, out=scales)
    return scales, trans
